# revision 1
# baseline (speedup 1.0000x reference)
"""Causal single-head attention on 8 trn2 NeuronCores.

Sharding: core c handles batch c//2 and half the query rows of that batch
(4 blocks of 256 rows, picked so causal work balances). The device program is
identical on every core; which rows a core owns is data (host-side
gather/scatter + per-core causal masks).

Algorithm (v2) — projections folded away:
  scores = x (Wq^T Wk) x^T and out = P x Wv, so the device never forms
  Q, K, or V:
    host:  A = Wq^T @ Wk  (f32)
    dev:   T^T = A^T x^T  over own queries            (xA)
           per query-block: S^T[j,i] = x^T.T_tiles @ T^T   (PSUM)
             + causal mask add, probsT = exp(S^T/32)  (no max needed:
               scaled scores are O(+-2))
           U^T[d,i] += x_nat_tile.T @ probsT   accumulated over j-tiles
           l[1,i]   += ones.T @ probsT         (softmax denominator)
           U^T /= l (broadcast) -> sbuf, then out = (U^T).T @ Wv^T tiles.
"""

import sys

try:
    import concourse  # noqa: F401
except ImportError:
    sys.path.insert(0, "/opt/trn_rl_repo")

from contextlib import ExitStack

import ml_dtypes
import numpy as np

import concourse.bass as bass
from concourse import bacc
import concourse.mybir as mybir
import concourse.tile as tile
from concourse.bass_utils import run_bass_kernel_spmd

B, N, D = 4, 2048, 1024
NQ = 1024            # query rows owned per core
NCORES = 8
TRIPS = (4, 8, 12, 16)          # j-tile trip count per slot (uniform program)
SLOTS = ((0, 2, 4, 6), (1, 3, 5, 7))  # 256-row block owned by slot s, per h
SCALE = 1.0 / 32.0   # 1/sqrt(D)
IB = 256             # query block width
MDT = mybir.dt.bfloat16
NPDT = ml_dtypes.bfloat16

TRACE = False
LAST_RESULT = None
LAST_IN_MAPS = None
_CACHED_NC = None


def _qrows(h):
    return np.concatenate([np.arange(256 * p, 256 * p + 256) for p in SLOTS[h]])


def _build_masks(h):
    """[4 slots, 4, 128, 256] f32: additive causal masks for the last 4 j-tiles
    of each slot (covers the diagonal tiles and the padded tiles)."""
    masks = np.zeros((4, 4, 128, IB), np.float32)
    jp = np.arange(128)[:, None]
    iv = np.arange(IB)[None, :]
    for s in range(4):
        r0 = 256 * SLOTS[h][s]
        for k in range(4):
            jt = TRIPS[s] - 4 + k
            masks[s, k] = np.where(jt * 128 + jp <= r0 + iv, 0.0, -1e30)
    return masks


def _build_body(nc, tc, ctx, dram, rep):
    P = 128
    n_d = D // P          # 8
    n_j = N // P          # 16
    xt_w = N + NQ
    xt_d, xn_d, a_d, wvt_d, mask_d, out_d = dram
    r = rep

    pool_xt = ctx.enter_context(tc.tile_pool(name=f"xt{r}", bufs=4 * n_d))
    pool_xq = ctx.enter_context(tc.tile_pool(name=f"xq{r}", bufs=n_d))
    pool_xn = ctx.enter_context(tc.tile_pool(name=f"xn{r}", bufs=n_j))
    pool_a = ctx.enter_context(tc.tile_pool(name=f"a{r}", bufs=n_d))
    pool_wv = ctx.enter_context(tc.tile_pool(name=f"wv{r}", bufs=n_d))
    pool_tt = ctx.enter_context(tc.tile_pool(name=f"tt{r}", bufs=n_d))
    pool_mask = ctx.enter_context(tc.tile_pool(name=f"mask{r}", bufs=16))
    pool_probs = ctx.enter_context(tc.tile_pool(name=f"probs{r}", bufs=16))
    pool_ut = ctx.enter_context(tc.tile_pool(name=f"ut{r}", bufs=2 * n_d))
    pool_lr = ctx.enter_context(tc.tile_pool(name=f"lr{r}", bufs=4))
    pool_out = ctx.enter_context(tc.tile_pool(name=f"outb{r}", bufs=2))
    pool_one = ctx.enter_context(tc.tile_pool(name=f"one{r}", bufs=1))

    # ---- loads ----
    ats = []
    for dt in range(n_d):
        t = pool_a.tile([P, D], MDT, tag="a", name=f"at{r}_{dt}")
        nc.scalar.dma_start(out=t, in_=a_d[dt * P:(dt + 1) * P, :])
        ats.append(t)
    xqs = []
    for dt in range(n_d):
        t = pool_xq.tile([P, NQ], MDT, tag="xq", name=f"xqt{r}_{dt}")
        nc.sync.dma_start(out=t, in_=xt_d[dt * P:(dt + 1) * P, N:])
        xqs.append(t)
    # key tiles [dt][jc]: [128, 512] each (4 j-chunks); DMA in first-use order
    xtk = [[None] * 4 for _ in range(n_d)]
    xns = [None] * n_j
    mask_tiles = [[None] * 4 for _ in range(4)]
    wvs = [None] * n_d

    def load_keys(jc):
        for dt in range(n_d):
            t = pool_xt.tile([P, 512], MDT, tag="xt", name=f"xtt{r}_{dt}_{jc}")
            nc.sync.dma_start(out=t, in_=xt_d[dt * P:(dt + 1) * P,
                                             jc * 512:(jc + 1) * 512])
            xtk[dt][jc] = t

    def load_xn(jc):
        for jt in range(4 * jc, 4 * jc + 4):
            t = pool_xn.tile([P, D], MDT, tag="xn", name=f"xnt{r}_{jt}")
            nc.sync.dma_start(out=t, in_=xn_d[jt * P:(jt + 1) * P, :])
            xns[jt] = t

    load_keys(0)
    load_xn(0)
    for s in range(4):
        for k in range(4):
            t = pool_mask.tile([P, IB], MDT, tag="mask",
                               name=f"mask{r}_{s}_{k}")
            nc.scalar.dma_start(out=t, in_=mask_d[s, k, :, :])
            mask_tiles[s][k] = t
    for dt in range(n_d):
        t = pool_wv.tile([P, D], MDT, tag="wv", name=f"wvt{r}_{dt}")
        nc.scalar.dma_start(out=t, in_=wvt_d[dt * P:(dt + 1) * P, :])
        wvs[dt] = t
    for jc in range(1, 4):
        load_keys(jc)
        load_xn(jc)
    ones = pool_one.tile([P, 1], MDT, tag="one", name=f"ones{r}")
    nc.vector.memset(ones, 1.0)

    # ---- phase 1: T^T[d2, i] = sum_d1 A[d1, d2] x^T[d1, i] over own queries
    tts = [pool_tt.tile([P, NQ], MDT, tag="tt", name=f"ttt{r}_{i}")
           for i in range(n_d)]
    with tc.tile_pool(name=f"ps1_{r}", bufs=8, space="PSUM") as ps1:
        for c0 in range(0, NQ, IB):
            for d2 in range(n_d):
                ps = ps1.tile([P, IB], mybir.dt.float32, tag="ps1",
                              name=f"pst{r}_{d2}_{c0}")
                for d1 in range(n_d):
                    nc.tensor.matmul(
                        ps,
                        lhsT=ats[d1][:, d2 * P:(d2 + 1) * P],
                        rhs=xqs[d1][:, c0:c0 + IB],
                        start=(d1 == 0), stop=(d1 == n_d - 1),
                    )
                nc.vector.tensor_copy(tts[d2][:, c0:c0 + IB], ps)

    # ---- phase 2: attention ----
    with (
        tc.tile_pool(name=f"ps_s{r}", bufs=3, space="PSUM") as ps_s,
        tc.tile_pool(name=f"ps_u{r}", bufs=2, space="PSUM") as ps_u,
        tc.tile_pool(name=f"ps_f{r}", bufs=2, space="PSUM") as ps_f,
        tc.tile_pool(name=f"ps_l{r}", bufs=1, space="PSUM") as ps_l,
    ):
        for s in range(4):
            trips = TRIPS[s]
            # pass 1: scores + exp; probs tiles persist for the slot
            probs_tiles = []
            for jt in range(trips):
                pss = ps_s.tile([P, IB], mybir.dt.float32, tag="pss",
                                name=f"pss{r}_{s}_{jt}")
                for d2 in range(n_d):
                    nc.tensor.matmul(
                        pss,
                        lhsT=xtk[d2][jt // 4][:, (jt % 4) * P:(jt % 4 + 1) * P],
                        rhs=tts[d2][:, s * IB:(s + 1) * IB],
                        start=(d2 == 0), stop=(d2 == n_d - 1),
                    )
                k = jt - (trips - 4)
                if k >= 0:
                    nc.vector.tensor_add(pss, pss, mask_tiles[s][k])
                probs = pool_probs.tile([P, IB], MDT, tag="probs",
                                        name=f"probs{r}_{s}_{jt}")
                nc.scalar.activation(probs, pss,
                                     mybir.ActivationFunctionType.Exp,
                                     scale=SCALE)
                probs_tiles.append(probs)

            # pass 2: U^T[d-tile, i] = sum_jt xn_tile.T @ probs, one bank
            # at a time so each accumulation group owns its bank exclusively.
            # The l^T (softmax denominator) matmuls are interleaved so their
            # LDWEIGHTS hide under the U streams in the PE reorder window.
            psl2 = ps_l.tile([P, 2], mybir.dt.float32, tag="l",
                             name=f"psl{r}_{s}")
            psl = [psl2[:, 0:1], psl2[:, 1:2]]
            uts = []
            for dt in range(n_d):
                psu = ps_u.tile([P, IB], mybir.dt.float32, tag="u",
                                name=f"psu{r}_{s}_{dt}")
                for jt in range(trips):
                    nc.tensor.matmul(
                        psu,
                        lhsT=xns[jt][:, dt * P:(dt + 1) * P],
                        rhs=probs_tiles[jt],
                        start=(jt == 0), stop=(jt == trips - 1),
                    )
                    if dt < 2:
                        nc.tensor.matmul(
                            psl[dt],
                            lhsT=probs_tiles[jt][:, dt * P:(dt + 1) * P],
                            rhs=ones,
                            start=(jt == 0 and dt == 0),
                            stop=(jt == trips - 1),
                        )
                ut = pool_ut.tile([P, IB], MDT, tag="ut",
                                  name=f"ut{r}_{s}_{dt}")
                nc.vector.tensor_copy(ut, psu)
                uts.append(ut)

            # out[i, o'] = (sum_d U^T[d, i]^T Wv^T[d, o']) / l[i]
            for half in range(2):
                rt = pool_lr.tile([P, 1], mybir.dt.float32, tag="lr",
                                  name=f"lrec{r}_{s}_{half}")
                nc.vector.reciprocal(rt, psl[half])
                obh = pool_out.tile([P, D], mybir.dt.float32, tag="obh",
                                    name=f"obh{r}_{s}_{half}")
                for c0 in range(0, D, 512):
                    psf = ps_f.tile([P, 512], mybir.dt.float32, tag="f",
                                    name=f"psf{r}_{s}_{half}_{c0}")
                    for dt in range(n_d):
                        nc.tensor.matmul(
                            psf,
                            lhsT=uts[dt][:, half * P:(half + 1) * P],
                            rhs=wvs[dt][:, c0:c0 + 512],
                            start=(dt == 0), stop=(dt == n_d - 1),
                        )
                    nc.vector.tensor_scalar_mul(obh[:, c0:c0 + 512], psf, rt)
                    r0 = s * IB + half * P
                    nc.sync.dma_start(out=out_d[r0:r0 + P, c0:c0 + 512],
                                      in_=obh[:, c0:c0 + 512])


def _build_nc(reps=1):
    nc = bacc.Bacc(None, target_bir_lowering=False)
    P = 128
    xt_w = N + NQ

    xt_d = nc.declare_dram_parameter("xt", [D, xt_w], MDT, isOutput=False)
    xn_d = nc.declare_dram_parameter("xn", [N, D], MDT, isOutput=False)
    a_d = nc.declare_dram_parameter("a", [D, D], MDT, isOutput=False)
    wvt_d = nc.declare_dram_parameter("wvt", [D, D], MDT, isOutput=False)
    mask_d = nc.declare_dram_parameter("masks", [4, 4, P, IB], MDT,
                                       isOutput=False)
    out_d = nc.declare_dram_parameter("out_p", [NQ, D], mybir.dt.float32,
                                      isOutput=True)
    dram = (xt_d, xn_d, a_d, wvt_d, mask_d, out_d)

    with tile.TileContext(nc) as tc:
        for rep in range(reps):
            with ExitStack() as ctx:
                _build_body(nc, tc, ctx, dram, rep)
    nc.finalize()
    return nc


def _make_in_maps(x, W_q, W_k, W_v):
    wq = np.asarray(W_q, np.float32)
    wk = np.asarray(W_k, np.float32)
    wv = np.asarray(W_v, np.float32)
    a = (wq.T @ wk).astype(NPDT)                       # [d1, d2]
    wvt = np.ascontiguousarray(wv.T).astype(NPDT)      # [d, o]
    masks = [_build_masks(0), _build_masks(1)]
    qrows = [_qrows(0), _qrows(1)]
    in_maps = []
    for c in range(NCORES):
        b, h = c // 2, c % 2
        xb = x[b]
        xb_t = xb.T  # [D, N]
        xt_all = np.concatenate([xb_t, xb_t[:, qrows[h]]], axis=1)
        in_maps.append({
            "xt": np.ascontiguousarray(xt_all).astype(NPDT),
            "xn": xb.astype(NPDT),
            "a": a, "wvt": wvt,
            "masks": masks[h].astype(NPDT),
        })
    return in_maps


def kernel(x, W_q, W_k, W_v):
    global _CACHED_NC, LAST_RESULT, LAST_IN_MAPS
    x = np.asarray(x, dtype=np.float32)
    if _CACHED_NC is None:
        _CACHED_NC = _build_nc()
    nc = _CACHED_NC

    in_maps = _make_in_maps(x, W_q, W_k, W_v)
    LAST_IN_MAPS = in_maps
    try:
        res = run_bass_kernel_spmd(nc, in_maps, list(range(NCORES)))
    except Exception:
        # transient NRT_EXEC_UNIT_UNRECOVERABLE wedges clear on retry
        import time as _time
        _time.sleep(5)
        res = run_bass_kernel_spmd(nc, in_maps, list(range(NCORES)))
    LAST_RESULT = res

    qrows = [_qrows(0), _qrows(1)]
    out = np.empty((B, N, D), np.float32)
    for c in range(NCORES):
        b, h = c // 2, c % 2
        out[b, qrows[h], :] = res.results[c]["out_p"]
    return out



# revision 39
# speedup vs baseline: 1.1844x; 1.1844x over previous
"""Causal single-head attention on 8 trn2 NeuronCores — fp8 DoubleRow version.

Sharding: core c handles batch c//2 and half the query rows of that batch
(4 blocks of 256 rows, picked so causal work balances). The device program is
identical on every core; which rows a core owns is data (host-side
gather/scatter + per-core causal masks).

Algorithm (v3) — projections folded away, all GEMMs in fp8e4m3 DoubleRow
perf mode with 3-term hi/lo error compensation (A*B ~= Ah*Bh + Al*Bh + Ah*Bl,
each operand split as M = fp8(M) + fp8(M - fp8(M))):
    host:  A = Wq^T @ Wk, all inputs split hi/lo and laid out "paired":
           [128, npair, 2, W] where (pair k, parity e) holds contraction
           rows 256k+128e .. 256k+128e+127 (DoubleRow consumes [:, :, slice]
           shaped [128, 2, F] = 256 contraction rows per instruction).
    dev:   T^T = A^T x^T over own queries (quantized to fp8 hi/lo on the fly)
           per query-block: S^T[j,i] = x^T_tiles . T^T  (PSUM, f32)
             + causal mask add, probs = exp(S^T/32) -> bf16 -> fp8 hi/lo
           U^T[d,i] += xn_tile^T . probs  (fp8 3-term), quantized to hi/lo
           l[1,i]   += probs^T . ones     (softmax denominator, exact f32)
           out = (U^T)^T . Wv^T * (1/l)   -> bf16 -> DRAM.
"""

import sys

try:
    import concourse  # noqa: F401
except ImportError:
    sys.path.insert(0, "/opt/trn_rl_repo")

from contextlib import ExitStack

import ml_dtypes
import numpy as np

import concourse.bass as bass
from concourse import bacc
import concourse.mybir as mybir
import concourse.tile as tile
from concourse.bass_utils import run_bass_kernel_spmd

B, N, D = 4, 2048, 1024
NQ = 1024            # query rows owned per core
NCORES = 8
TRIPS = (4, 8, 12, 16)          # j-tile trip count per slot (uniform program)
SLOTS = ((0, 2, 4, 6), (1, 3, 5, 7))  # 256-row block owned by slot s, per h
SCALE = 1.0 / 32.0   # 1/sqrt(D)
SA = 64.0            # host pre-scale on A and Wv^T: their raw entries sit in
                     # fp8e4m3's subnormal range (~0.01-0.02 < 2^-6) which
                     # wrecks the hi/lo compensation; x64 moves them to the
                     # normal range. Compensated via exp scale and `ones`.
EXP_SCALE = SCALE / SA       # scores PSUM holds 64*S
SU_INV = 1.0 / 16.0          # U quantized as U/16 (avoids fp8 overflow)
ONES_VAL = SA * SU_INV       # denominator pre-scale so rt = 1/(4*sum p)
IB = 256             # query block width
P = 128
F8 = mybir.dt.float8e4
NF8 = ml_dtypes.float8_e4m3
BF16 = mybir.dt.bfloat16
F32 = mybir.dt.float32
DR = mybir.MatmulPerfMode.DoubleRow

TRACE = False
LAST_RESULT = None
LAST_IN_MAPS = None
_CACHED_NC = None


def _qrows(h):
    return np.concatenate([np.arange(256 * p, 256 * p + 256) for p in SLOTS[h]])


def _build_masks(h):
    """[4 slots, 128, 4, 256] bf16: additive causal masks for the last 4
    j-tiles of each slot (diagonal + padded tiles), laid out so one DMA per
    slot loads a [128, 4, 256] SBUF tile."""
    masks = np.zeros((4, P, 4, IB), np.float32)
    jp = np.arange(P)[:, None]
    iv = np.arange(IB)[None, :]
    for s in range(4):
        r0 = 256 * SLOTS[h][s]
        for k in range(4):
            jt = TRIPS[s] - 4 + k
            masks[s, :, k, :] = np.where(jt * P + jp <= r0 + iv, 0.0, -1e30)
    return masks


def _build_body(nc, tc, ctx, dram, rep):
    n_d = D // P          # 8 d-chunks, 4 pairs
    r = rep
    (xk_h_d, xk_l_d, xq_h_d, xq_l_d, xn_h_d, xn_l_d,
     a_h_d, a_l_d, wv_h_d, wv_l_d, mask_d, out_d) = dram

    pool_a = ctx.enter_context(tc.tile_pool(name=f"a{r}", bufs=16))
    pool_xq = ctx.enter_context(tc.tile_pool(name=f"xq{r}", bufs=8))
    pool_xk = ctx.enter_context(tc.tile_pool(name=f"xk{r}", bufs=8))
    pool_xn = ctx.enter_context(tc.tile_pool(name=f"xn{r}", bufs=8))
    pool_wv = ctx.enter_context(tc.tile_pool(name=f"wv{r}", bufs=2))
    pool_t = ctx.enter_context(tc.tile_pool(name=f"t{r}", bufs=32))
    pool_mask = ctx.enter_context(tc.tile_pool(name=f"mask{r}", bufs=4))
    pool_pb = ctx.enter_context(tc.tile_pool(name=f"pb{r}", bufs=10))
    pool_p8 = ctx.enter_context(tc.tile_pool(name=f"p8{r}", bufs=20))
    pool_u = ctx.enter_context(tc.tile_pool(name=f"u{r}", bufs=16))
    pool_lr = ctx.enter_context(tc.tile_pool(name=f"lr{r}", bufs=4))
    pool_out = ctx.enter_context(tc.tile_pool(name=f"outb{r}", bufs=4))
    pool_one = ctx.enter_context(tc.tile_pool(name=f"one{r}", bufs=1))

    # ---- SBUF tiles: one tile per DMA (dependency tracking is per-tile).
    # Host lays every tensor out so a whole logical block is one contiguous
    # DMA: a/wv [P,4,2,D]; xq [P,4(c0),4,2,256]; xk [P,4(jc),4,2,512];
    # xn [P,8,2,D] loaded per 2-pair group.
    aHd = [pool_a.tile([P, 4, 2, IB], F8, tag="a", name=f"aH{r}_{c}")
           for c in range(4)]
    aLd = [pool_a.tile([P, 4, 2, IB], F8, tag="a", name=f"aL{r}_{c}")
           for c in range(4)]
    xqH = [pool_xq.tile([P, 4, 2, IB], F8, tag="xq", name=f"xqH{r}_{c}")
           for c in range(4)]
    xqL = [pool_xq.tile([P, 4, 2, IB], F8, tag="xq", name=f"xqL{r}_{c}")
           for c in range(4)]
    xkH = [pool_xk.tile([P, 4, 2, 512], F8, tag="xk", name=f"xkH{r}_{jc}")
           for jc in range(4)]
    xkL = [pool_xk.tile([P, 4, 2, 512], F8, tag="xk", name=f"xkL{r}_{jc}")
           for jc in range(4)]
    # xn group g holds j-pairs 2g, 2g+1 (j-tiles 4g..4g+3)
    xnH = [pool_xn.tile([P, 2, 2, D], F8, tag="xn", name=f"xnH{r}_{g}")
           for g in range(4)]
    xnL = [pool_xn.tile([P, 2, 2, D], F8, tag="xn", name=f"xnL{r}_{g}")
           for g in range(4)]
    wvH = pool_wv.tile([P, 4, 2, D], F8, tag="wv", name=f"wvH{r}")
    wvL = pool_wv.tile([P, 4, 2, D], F8, tag="wv", name=f"wvL{r}")
    # T: per (pair, slot) tiles so slot-s scores wait only on their block
    th = [[pool_t.tile([P, 2, IB], F8, tag="t", name=f"th{r}_{k}_{s}")
           for s in range(4)] for k in range(4)]
    tl = [[pool_t.tile([P, 2, IB], F8, tag="t", name=f"tl{r}_{k}_{s}")
           for s in range(4)] for k in range(4)]
    mask_tiles = [pool_mask.tile([P, 4, IB], BF16, tag="mask",
                                 name=f"mask{r}_{s}") for s in range(4)]

    def a_slice(hi, k, d2):
        o = (d2 % 2) * P
        return (aHd if hi else aLd)[d2 // 2][:, k, :, o:o + P]

    def xq_slice(hi, k, c0):
        t = (xqH if hi else xqL)[c0 // IB]
        return t[:, k, :, :]

    # ---- DMA schedule (consumption order, all on the SP queue; Act/DVE
    # carry no DMAs so compute chains never queue behind DMA issue).
    # Phase-1 processes c0-blocks in order 1,2,3,0 matching this stream.
    nc.sync.dma_start(out=aHd[0], in_=a_h_d[:, 0, :, :, :])
    nc.sync.dma_start(out=xqH[1], in_=xq_h_d[:, 1, :, :, :])
    nc.sync.dma_start(out=aLd[0], in_=a_l_d[:, 0, :, :, :])
    nc.sync.dma_start(out=xqL[1], in_=xq_l_d[:, 1, :, :, :])
    for c in range(1, 4):
        nc.sync.dma_start(out=aHd[c], in_=a_h_d[:, c, :, :, :])
        nc.sync.dma_start(out=aLd[c], in_=a_l_d[:, c, :, :, :])
    for c in (2, 3, 0):
        nc.sync.dma_start(out=xqH[c], in_=xq_h_d[:, c, :, :, :])
        nc.sync.dma_start(out=xqL[c], in_=xq_l_d[:, c, :, :, :])

    def load_xk(jc):
        nc.sync.dma_start(out=xkH[jc], in_=xk_h_d[:, jc, :, :, :])
        nc.sync.dma_start(out=xkL[jc], in_=xk_l_d[:, jc, :, :, :])

    def load_xn(g):
        gs = slice(2 * g, 2 * g + 2)
        nc.sync.dma_start(out=xnH[g], in_=xn_h_d[:, gs, :, :])
        nc.sync.dma_start(out=xnL[g], in_=xn_l_d[:, gs, :, :])

    load_xk(0)
    nc.sync.dma_start(out=mask_tiles[0], in_=mask_d[0, :, :, :])
    load_xn(0)
    nc.sync.dma_start(out=wvH, in_=wv_h_d[:, :, :, :])
    nc.sync.dma_start(out=wvL, in_=wv_l_d[:, :, :, :])
    nc.sync.dma_start(out=mask_tiles[1], in_=mask_d[1, :, :, :])
    load_xk(1)
    load_xn(1)
    nc.sync.dma_start(out=mask_tiles[2], in_=mask_d[2, :, :, :])
    load_xk(2)
    load_xn(2)
    nc.sync.dma_start(out=mask_tiles[3], in_=mask_d[3, :, :, :])
    load_xk(3)
    load_xn(3)

    ones = pool_one.tile([P, 2, 1], F8, tag="one", name=f"ones{r}")
    nc.vector.memset(ones, ONES_VAL)

    # ---- phase 1: T[d2, i] = sum_d1 A[d1, d2] x^T[d1, i] over own queries,
    # 3-term fp8 DoubleRow, quantized to th/tl as each [128, 256] lands.
    with tc.tile_pool(name=f"ps1_{r}", bufs=4, space="PSUM") as ps1:
        for s in (1, 2, 3, 0):
            c0 = s * IB
            for d2 in range(n_d):
                ps = ps1.tile([P, IB], F32, tag="ps1", name=f"pst{r}_{d2}_{c0}")
                for k in range(4):
                    nc.tensor.matmul(ps, lhsT=a_slice(True, k, d2),
                                     rhs=xq_slice(True, k, c0),
                                     start=(k == 0), stop=False, perf_mode=DR)
                    nc.tensor.matmul(ps, lhsT=a_slice(False, k, d2),
                                     rhs=xq_slice(True, k, c0),
                                     start=False, stop=False, perf_mode=DR)
                    nc.tensor.matmul(ps, lhsT=a_slice(True, k, d2),
                                     rhs=xq_slice(False, k, c0),
                                     start=False, stop=(k == 3), perf_mode=DR)
                hs = th[d2 // 2][s][:, d2 % 2, :]
                nc.scalar.activation(hs, ps, mybir.ActivationFunctionType.Copy)
                nc.vector.tensor_sub(tl[d2 // 2][s][:, d2 % 2, :], ps, hs)

    # ---- phase 2: attention ----
    with (
        tc.tile_pool(name=f"ps_s{r}", bufs=3, space="PSUM") as ps_s,
        tc.tile_pool(name=f"ps_u{r}", bufs=2, space="PSUM") as ps_u,
        tc.tile_pool(name=f"ps_f{r}", bufs=2, space="PSUM") as ps_f,
        tc.tile_pool(name=f"ps_l{r}", bufs=1, space="PSUM") as ps_l,
    ):
        for s in (1, 2, 3, 0):
            trips = TRIPS[s]
            qs = slice(s * IB, (s + 1) * IB)
            # pass A: scores + exp + fp8 split; pair tiles persist for slot
            pbs, phs, pls = [], [], []
            for jt in range(trips):
                m = jt // 2
                if jt % 2 == 0:
                    pbs.append(pool_pb.tile([P, 2, IB], BF16, tag="pb",
                                            name=f"pb{r}_{s}_{m}"))
                    phs.append(pool_p8.tile([P, 2, IB], F8, tag="p8",
                                            name=f"ph{r}_{s}_{m}"))
                    pls.append(pool_p8.tile([P, 2, IB], F8, tag="p8",
                                            name=f"pl{r}_{s}_{m}"))
                pss = ps_s.tile([P, IB], F32, tag="pss",
                                name=f"pss{r}_{s}_{jt}")
                jc, jo = jt // 4, (jt % 4) * P
                jtile = slice(jo, jo + P)
                for k in range(4):
                    nc.tensor.matmul(pss, lhsT=xkH[jc][:, k, :, jtile],
                                     rhs=th[k][s],
                                     start=(k == 0), stop=False, perf_mode=DR)
                    nc.tensor.matmul(pss, lhsT=xkL[jc][:, k, :, jtile],
                                     rhs=th[k][s],
                                     start=False, stop=False, perf_mode=DR)
                    nc.tensor.matmul(pss, lhsT=xkH[jc][:, k, :, jtile],
                                     rhs=tl[k][s],
                                     start=False, stop=(k == 3), perf_mode=DR)
                kk = jt - (trips - 4)
                if kk >= 0:
                    nc.vector.tensor_add(pss, pss, mask_tiles[s][:, kk, :])
                nc.scalar.activation(pbs[m][:, jt % 2, :], pss,
                                     mybir.ActivationFunctionType.Exp,
                                     scale=EXP_SCALE)
                if jt % 2 == 1:
                    nc.gpsimd.tensor_copy(phs[m], pbs[m])
                    nc.vector.tensor_sub(pls[m], pbs[m], phs[m])

            # pass B: U^T accumulation + softmax denominator, fp8 3-term
            psl2 = ps_l.tile([P, 2], F32, tag="l", name=f"psl{r}_{s}")
            psl = [psl2[:, 0:1], psl2[:, 1:2]]
            uh = [pool_u.tile([P, 2, IB], F8, tag="u", name=f"uh{r}_{s}_{k}")
                  for k in range(4)]
            ul = [pool_u.tile([P, 2, IB], F8, tag="u", name=f"ul{r}_{s}_{k}")
                  for k in range(4)]
            npair = trips // 2
            for dt in range(n_d):
                ds = slice(dt * P, (dt + 1) * P)
                psu = ps_u.tile([P, IB], F32, tag="u", name=f"psu{r}_{s}_{dt}")
                for m in range(npair):
                    g, o = m // 2, m % 2
                    xh = xnH[g][:, o, :, ds]
                    xl = xnL[g][:, o, :, ds]
                    nc.tensor.matmul(psu, lhsT=xh, rhs=phs[m],
                                     start=(m == 0), stop=False, perf_mode=DR)
                    nc.tensor.matmul(psu, lhsT=xl, rhs=phs[m],
                                     start=False, stop=False, perf_mode=DR)
                    nc.tensor.matmul(psu, lhsT=xh, rhs=pls[m],
                                     start=False, stop=(m == npair - 1),
                                     perf_mode=DR)
                    if dt < 2:
                        hsl = slice(dt * P, dt * P + P)
                        nc.tensor.matmul(psl[dt], lhsT=phs[m][:, :, hsl],
                                         rhs=ones,
                                         start=(m == 0), stop=False,
                                         perf_mode=DR)
                        nc.tensor.matmul(psl[dt], lhsT=pls[m][:, :, hsl],
                                         rhs=ones,
                                         start=False, stop=(m == npair - 1),
                                         perf_mode=DR)
                hs = uh[dt // 2][:, dt % 2, :]
                nc.scalar.activation(hs, psu, mybir.ActivationFunctionType.Copy,
                                     scale=SU_INV)
                nc.vector.scalar_tensor_tensor(
                    ul[dt // 2][:, dt % 2, :], psu, SU_INV, hs,
                    mybir.AluOpType.mult, mybir.AluOpType.subtract)

            # pass C: out[i, o] = ((U^T)^T Wv^T)[i, o] / l[i]  -> bf16 DRAM
            for half in range(2):
                rt = pool_lr.tile([P, 1], F32, tag="lr",
                                  name=f"lrec{r}_{s}_{half}")
                nc.vector.reciprocal(rt, psl[half])
                hsl = slice(half * P, half * P + P)
                if s == 0 and half == 1:
                    chunks = [(0, 512), (512, 896), (896, 1024)]
                else:
                    chunks = [(0, 512), (512, 1024)]
                for c0, c1 in chunks:
                    cs = slice(c0, c1)
                    psf = ps_f.tile([P, c1 - c0], F32, tag="f",
                                    name=f"psf{r}_{s}_{half}_{c0}")
                    for k in range(4):
                        nc.tensor.matmul(psf, lhsT=uh[k][:, :, hsl],
                                         rhs=wvH[:, k, :, cs],
                                         start=(k == 0), stop=False,
                                         perf_mode=DR)
                        nc.tensor.matmul(psf, lhsT=ul[k][:, :, hsl],
                                         rhs=wvH[:, k, :, cs],
                                         start=False, stop=False, perf_mode=DR)
                        nc.tensor.matmul(psf, lhsT=uh[k][:, :, hsl],
                                         rhs=wvL[:, k, :, cs],
                                         start=False, stop=(k == 3),
                                         perf_mode=DR)
                    obh = pool_out.tile([P, c1 - c0], BF16, tag="obh",
                                        name=f"obh{r}_{s}_{half}_{c0}")
                    nc.scalar.activation(obh, psf,
                                         mybir.ActivationFunctionType.Copy,
                                         scale=rt)
                    r0 = s * IB + half * P
                    # final slot's stores ride the (then-idle) Act queue so
                    # the drain tail isn't serialized behind SP DMA issue
                    dq = nc.scalar if s == 0 else nc.sync
                    dq.dma_start(out=out_d[r0:r0 + P, cs], in_=obh)


def _build_nc(reps=1):
    nc = bacc.Bacc(None, target_bir_lowering=False)

    def dp(name, shape, dtype):
        return nc.declare_dram_parameter(name, shape, dtype, isOutput=False)

    xk_h_d = dp("xk_h", [P, 4, 4, 2, 512], F8)
    xk_l_d = dp("xk_l", [P, 4, 4, 2, 512], F8)
    xq_h_d = dp("xq_h", [P, 4, 4, 2, IB], F8)
    xq_l_d = dp("xq_l", [P, 4, 4, 2, IB], F8)
    xn_h_d = dp("xn_h", [P, 8, 2, D], F8)
    xn_l_d = dp("xn_l", [P, 8, 2, D], F8)
    a_h_d = dp("a_h", [P, 8, 4, 2, P], F8)
    a_l_d = dp("a_l", [P, 8, 4, 2, P], F8)
    wv_h_d = dp("wv_h", [P, 4, 2, D], F8)
    wv_l_d = dp("wv_l", [P, 4, 2, D], F8)
    mask_d = dp("masks", [4, P, 4, IB], BF16)
    out_d = nc.declare_dram_parameter("out_p", [NQ, D], BF16, isOutput=True)
    dram = (xk_h_d, xk_l_d, xq_h_d, xq_l_d, xn_h_d, xn_l_d,
            a_h_d, a_l_d, wv_h_d, wv_l_d, mask_d, out_d)

    with tile.TileContext(nc) as tc:
        for rep in range(reps):
            with ExitStack() as ctx:
                _build_body(nc, tc, ctx, dram, rep)
    nc.finalize()
    return nc


def _split8(x):
    """f32 array -> (hi, lo) fp8e4m3 arrays with x ~= hi + lo."""
    hi = x.astype(NF8)
    lo = (x - hi.astype(np.float32)).astype(NF8)
    return hi, lo


def _pair_d(x):
    """[1024(contraction), W] -> paired [128, 4, 2, W]."""
    w = x.shape[1]
    return np.ascontiguousarray(
        x.reshape(4, 2, P, w).transpose(2, 0, 1, 3))


def _pair_d_chunked(x, cw):
    """[1024(contraction), W] -> [128, W//cw, 4, 2, cw]: paired layout with
    the free dim chunked outermost so one chunk is one contiguous DMA."""
    w = x.shape[1]
    p = x.reshape(4, 2, P, w // cw, cw)
    return np.ascontiguousarray(p.transpose(2, 3, 0, 1, 4))


def _pair_j(x):
    """[2048(contraction), W] -> paired [128, 8, 2, W]."""
    w = x.shape[1]
    return np.ascontiguousarray(
        x.reshape(8, 2, P, w).transpose(2, 0, 1, 3))


def _make_in_maps(x, W_q, W_k, W_v):
    wq = np.asarray(W_q, np.float32)
    wk = np.asarray(W_k, np.float32)
    wv = np.asarray(W_v, np.float32)
    a = (wq.T @ wk) * SA                               # [d1, d2], pre-scaled
    a_h, a_l = _split8(a)
    wvt_h, wvt_l = _split8(np.ascontiguousarray(wv.T) * SA)  # [d, o]
    a_h, a_l = _pair_d_chunked(a_h, P), _pair_d_chunked(a_l, P)
    wvt_h, wvt_l = _pair_d(wvt_h), _pair_d(wvt_l)
    masks = [_build_masks(0).astype(ml_dtypes.bfloat16),
             _build_masks(1).astype(ml_dtypes.bfloat16)]
    qrows = [_qrows(0), _qrows(1)]
    in_maps = []
    for c in range(NCORES):
        b, h = c // 2, c % 2
        xb = np.asarray(x[b], np.float32)
        xb_t = np.ascontiguousarray(xb.T)              # [D, N]
        xk_h, xk_l = _split8(xb_t)
        xq_h = np.ascontiguousarray(xk_h[:, qrows[h]])
        xq_l = np.ascontiguousarray(xk_l[:, qrows[h]])
        xn_h, xn_l = _split8(xb)
        in_maps.append({
            "xk_h": _pair_d_chunked(xk_h, 512),
            "xk_l": _pair_d_chunked(xk_l, 512),
            "xq_h": _pair_d_chunked(xq_h, IB),
            "xq_l": _pair_d_chunked(xq_l, IB),
            "xn_h": _pair_j(xn_h), "xn_l": _pair_j(xn_l),
            "a_h": a_h, "a_l": a_l,
            "wv_h": wvt_h, "wv_l": wvt_l,
            "masks": masks[h],
        })
    return in_maps


def kernel(x, W_q, W_k, W_v):
    global _CACHED_NC, LAST_RESULT, LAST_IN_MAPS
    x = np.asarray(x, dtype=np.float32)
    if _CACHED_NC is None:
        _CACHED_NC = _build_nc()
    nc = _CACHED_NC

    in_maps = _make_in_maps(x, W_q, W_k, W_v)
    LAST_IN_MAPS = in_maps
    try:
        res = run_bass_kernel_spmd(nc, in_maps, list(range(NCORES)))
    except Exception:
        # transient NRT_EXEC_UNIT_UNRECOVERABLE wedges clear on retry
        import time as _time
        _time.sleep(5)
        res = run_bass_kernel_spmd(nc, in_maps, list(range(NCORES)))
    LAST_RESULT = res

    qrows = [_qrows(0), _qrows(1)]
    out = np.empty((B, N, D), np.float32)
    for c in range(NCORES):
        b, h = c // 2, c % 2
        out[b, qrows[h], :] = res.results[c]["out_p"].astype(np.float32)
    return out
